# revision 9
# baseline (speedup 1.0000x reference)
"""Multi-head attention (B=2, S=2048, D=1024, H=16) on 8 Trainium2 cores.

Sharding: core c -> batch b = c // 4, head group g = c % 4 (4 heads each).
Each core computes its 4 heads end-to-end (QKV proj -> attention -> out-proj
partial) and returns a partial [S, D] output; the host sums the 4 partials
per batch and adds the output bias.

Per-core dataflow (all matmuls contract over the partition dim):
  X^T   : TensorE transpose of the core's [S, D] inputs, tile by tile
  Q^T/K^T/V^T = W^T @ X^T  (lhsT = weight slab, rhs = X^T)
  V~    : V^T transposed back to [keys, hd] with a ones column appended
  S^T   : K @ Q^T per head  ->  PSUM [keys, q]
  P     : exp(S^T / 8)      ->  ScalarE activation, PSUM -> SBUF
  U~    : V~^T @ P = [[P V]^T ; colsums]  (softmax numerator + denominator)
  O^T   : U~ rows 0..63 scaled by 1/denominator
  out   : O^T used as lhsT against Wo rows -> partial [S, D]
"""

import numpy as np

import concourse.bass as bass
import concourse.mybir as mybir
import concourse.tile as tile
from concourse import bacc
from concourse.bass import ts, ds
from concourse.bass_utils import run_bass_kernel_spmd
from concourse.masks import make_identity

F32 = mybir.dt.float32
F32R = mybir.dt.float32r

B, S, D = 2, 2048, 1024
H_TOT, HD = 16, 64
HC = 4                 # heads per core
DC = HC * HD           # 256 columns of QKV proj per core
NCORES = 8
P = 128
NDT = D // P           # 8 d-model tiles
NKT = S // P           # 16 key tiles
NQT = S // P           # 16 query tiles
CG = 1024              # q chunk width in attention
NCG = S // CG
SCALE = 1.0 / np.sqrt(HD)

# fp32 matmuls run at 4 cycles/row on the PE; float32r (same bits, reduced
# internal precision) runs at 1 cycle/row for moving dims >= 256.
USE_F32R = False


def _mm(ap):
    return ap.bitcast(F32R) if USE_F32R else ap


def _body(ctx, tc, xq, xk, xv, wq, wk, wv, bq, bk, bv, wo, outp):
    nc = tc.nc
    # DRAM bounce buffer for broadcasting softmax denominators across partitions
    rec_dram = nc.dram_tensor("rec_scratch", [8, CG], F32).ap()

    singles = ctx.enter_context(tc.tile_pool(name="singles", bufs=1))
    xpool = ctx.enter_context(tc.tile_pool(name="xpool", bufs=2))
    wpool = ctx.enter_context(tc.tile_pool(name="wpool", bufs=1))
    ppool = ctx.enter_context(tc.tile_pool(name="ppool", bufs=2))
    opool = ctx.enter_context(tc.tile_pool(name="opool", bufs=2))
    psA = ctx.enter_context(tc.tile_pool(name="psA", bufs=2, space="PSUM"))
    psU = ctx.enter_context(tc.tile_pool(name="psU", bufs=1, space="PSUM"))

    identity = singles.tile([P, P], F32, tag="ident")
    make_identity(nc, identity)

    # Persistent per-core tensors (partition dim x free dims)
    XT = singles.tile([P, NDT, S], F32, tag="xt")            # X^T, reused per input
    QT = [singles.tile([P, S], F32, tag=f"qt{m}", name=f"qt{m}") for m in range(2)]
    KT = [singles.tile([P, S], F32, tag=f"kt{m}", name=f"kt{m}") for m in range(2)]
    VT = [singles.tile([P, S], F32, tag=f"vt{m}", name=f"vt{m}") for m in range(2)]
    OT = [singles.tile([P, S], F32, tag=f"ot{m}", name=f"ot{m}") for m in range(2)]
    Vt = singles.tile([P, NKT, HC, HD + 1], F32, tag="vtile")  # [keys, kt, h, hd+1]

    wo_sb = singles.tile([P, 2, D], F32, tag="wo")
    nc.sync.dma_start(out=wo_sb, in_=wo.rearrange("(k p) d -> p k d", p=P))

    # ---- projections: for each input, transpose X then compute W^T @ X^T ----
    for x_dram, w_dram, b_dram, DEST in (
        (xq, wq, bq, QT),
        (xk, wk, bk, KT),
        (xv, wv, bv, VT),
    ):
        w_sb = wpool.tile([P, NDT, DC], F32, tag="w")
        nc.sync.dma_start(out=w_sb, in_=w_dram.rearrange("(t p) c -> p t c", p=P))
        b_sb = wpool.tile([P, 2], F32, tag="b")
        nc.sync.dma_start(out=b_sb, in_=b_dram.rearrange("(m p) -> p m", p=P))

        for qt in range(NQT):
            x_sb = xpool.tile([P, D], F32, tag="x")
            nc.sync.dma_start(out=x_sb, in_=x_dram[ts(qt, P), :])
            tp = psA.tile([P, D], F32, tag="mm")
            for dt in range(NDT):
                nc.tensor.transpose(tp[:, ts(dt, P)], x_sb[:, ts(dt, P)], identity)
            nc.vector.tensor_copy(
                out=XT[:, :, ts(qt, P)],
                in_=tp.rearrange("p (t q) -> p t q", q=P),
            )

        for m in range(2):
            for cg in range(NCG):
                ps = psA.tile([P, CG], F32, tag="mm")
                for dt in range(NDT):
                    for c2 in range(2):
                        nc.tensor.matmul(
                            ps[:, ts(c2, 512)],
                            lhsT=_mm(w_sb[:, dt, ts(m, P)]),
                            rhs=_mm(XT[:, dt, ds(cg * CG + c2 * 512, 512)]),
                            start=(dt == 0),
                            stop=(dt == NDT - 1),
                        )
                nc.vector.tensor_scalar_add(
                    out=DEST[m][:, ts(cg, CG)], in0=ps, scalar1=b_sb[:, m : m + 1]
                )

    # ---- build V~ = [V | ones] in [keys, hd+1] layout per (kt, head) ----
    nc.vector.memset(Vt[:, :, :, HD : HD + 1], 1.0)
    for h in range(HC):
        m, po = divmod(h, 2)
        for g in range(2):
            tp = psA.tile([P, 8 * HD], F32, tag="mm")
            for j in range(8):
                kt = g * 8 + j
                nc.tensor.transpose(
                    tp[:, ts(j, HD)],
                    VT[m][64 * po : 64 * po + 64, ts(kt, P)],
                    identity[64 * po : 64 * po + 64, 64 * po : 64 * po + 64],
                )
            nc.vector.tensor_copy(
                out=Vt[:, ds(g * 8, 8), h, 0:HD],
                in_=tp.rearrange("p (j q) -> p j q", q=HD),
            )

    # ---- attention: per head pair, per q chunk ----
    for hp in range(2):
        for cg in range(NCG):
            U = [psU.tile([HD + 1, CG], F32, tag=f"u{i}", name=f"u{i}") for i in range(2)]
            for kt in range(NKT):
                for i in range(2):
                    h = 2 * hp + i
                    po = 64 * i
                    s_ps = psA.tile([P, CG], F32, tag="mm")
                    for c2 in range(2):
                        nc.tensor.matmul(
                            s_ps[:, ts(c2, 512)],
                            lhsT=_mm(KT[hp][po : po + 64, ts(kt, P)]),
                            rhs=_mm(QT[hp][po : po + 64, ds(cg * CG + c2 * 512, 512)]),
                            start=True,
                            stop=True,
                        )
                    pe = ppool.tile([P, CG], F32, tag="pexp")
                    nc.scalar.activation(
                        out=pe,
                        in_=s_ps,
                        func=mybir.ActivationFunctionType.Exp,
                        scale=float(SCALE),
                    )
                    for c2 in range(2):
                        nc.tensor.matmul(
                            U[i][:, ts(c2, 512)],
                            lhsT=_mm(Vt[:, kt, h, :]),
                            rhs=_mm(pe[:, ts(c2, 512)]),
                            start=(kt == 0),
                            stop=(kt == NKT - 1),
                        )
            for i in range(2):
                idx = (hp * NCG + cg) * 2 + i
                rec = opool.tile([1, CG], F32, tag="rec")
                nc.vector.reciprocal(out=rec, in_=U[i][HD : HD + 1, :])
                nc.sync.dma_start(out=rec_dram[idx : idx + 1, :], in_=rec)
                bc = opool.tile([64, CG], F32, tag="bc")
                row = rec_dram[idx, :]
                bcast = bass.AP(
                    tensor=row.tensor, offset=row.offset, ap=[[0, 64]] + list(row.ap)
                )
                nc.gpsimd.dma_start(out=bc, in_=bcast)
                nc.vector.tensor_mul(
                    out=OT[hp][64 * i : 64 * i + 64, ts(cg, CG)],
                    in0=U[i][0:HD, :],
                    in1=bc,
                )

    # ---- out projection: partial = O @ Wo_slice ----
    for qt in range(NQT):
        ps = psA.tile([P, D], F32, tag="mm")
        for k2 in range(2):
            for c2 in range(2):
                nc.tensor.matmul(
                    ps[:, ts(c2, 512)],
                    lhsT=_mm(OT[k2][:, ts(qt, P)]),
                    rhs=_mm(wo_sb[:, k2, ts(c2, 512)]),
                    start=(k2 == 0),
                    stop=(k2 == 1),
                )
        ob = opool.tile([P, D], F32, tag="ob")
        nc.vector.tensor_copy(out=ob, in_=ps)
        nc.sync.dma_start(out=outp[ts(qt, P), :], in_=ob)


def build_nc():
    nc = bacc.Bacc("TRN2", target_bir_lowering=False, debug=False)
    aps = {}
    for name, shape in (
        ("xq", [S, D]),
        ("xk", [S, D]),
        ("xv", [S, D]),
        ("wq", [D, DC]),
        ("wk", [D, DC]),
        ("wv", [D, DC]),
        ("bq", [DC]),
        ("bk", [DC]),
        ("bv", [DC]),
        ("wo", [DC, D]),
    ):
        aps[name] = nc.dram_tensor(name, shape, F32, kind="ExternalInput").ap()
    aps["outp"] = nc.dram_tensor("out_partial", [S, D], F32, kind="ExternalOutput").ap()

    from contextlib import ExitStack

    with tile.TileContext(nc) as tc:
        with ExitStack() as ctx:
            _body(
                ctx,
                tc,
                aps["xq"], aps["xk"], aps["xv"],
                aps["wq"], aps["wk"], aps["wv"],
                aps["bq"], aps["bk"], aps["bv"],
                aps["wo"], aps["outp"],
            )
    nc.compile()
    return nc


def make_in_maps(inputs):
    q = np.asarray(inputs["query"], dtype=np.float32)
    k = np.asarray(inputs.get("key_", inputs.get("key")), dtype=np.float32)
    v = np.asarray(inputs["value"], dtype=np.float32)
    Wq = np.asarray(inputs["Wq"], dtype=np.float32)
    Wk = np.asarray(inputs["Wk"], dtype=np.float32)
    Wv = np.asarray(inputs["Wv"], dtype=np.float32)
    bq = np.asarray(inputs["bq"], dtype=np.float32)
    bk = np.asarray(inputs["bk"], dtype=np.float32)
    bv = np.asarray(inputs["bv"], dtype=np.float32)
    Wo = np.asarray(inputs["Wo"], dtype=np.float32)

    in_maps = []
    for c in range(NCORES):
        b, g = divmod(c, 4)
        cs = slice(DC * g, DC * (g + 1))
        in_maps.append(
            {
                "xq": np.ascontiguousarray(q[b]),
                "xk": np.ascontiguousarray(k[b]),
                "xv": np.ascontiguousarray(v[b]),
                "wq": np.ascontiguousarray(Wq[:, cs]),
                "wk": np.ascontiguousarray(Wk[:, cs]),
                "wv": np.ascontiguousarray(Wv[:, cs]),
                "bq": np.ascontiguousarray(bq[cs]),
                "bk": np.ascontiguousarray(bk[cs]),
                "bv": np.ascontiguousarray(bv[cs]),
                "wo": np.ascontiguousarray(Wo[cs, :]),
            }
        )
    return in_maps


_NC_CACHE = {}


def get_nc():
    if "nc" not in _NC_CACHE:
        _NC_CACHE["nc"] = build_nc()
    return _NC_CACHE["nc"]


def kernel(**inputs):
    nc = get_nc()
    in_maps = make_in_maps(inputs)
    res = run_bass_kernel_spmd(nc, in_maps, list(range(NCORES))).results
    bo = np.asarray(inputs["bo"], dtype=np.float32)
    out = np.empty((B, S, D), dtype=np.float32)
    for b in range(B):
        acc = res[4 * b + 0]["out_partial"].astype(np.float32)
        for g in range(1, 4):
            acc = acc + res[4 * b + g]["out_partial"]
        out[b] = acc + bo[None, :]
    return out


# revision 12
# speedup vs baseline: 1.4679x; 1.4679x over previous
"""Multi-head attention (B=2, S=2048, D=1024, H=16) on 8 Trainium2 cores.

Sharding: core c -> batch b = c // 4, head group g = c % 4 (4 heads each).
Each core computes its 4 heads end-to-end (QKV proj -> attention -> out-proj
partial) and returns a partial [S, D] output; the host sums the 4 partials
per batch and adds the output bias.

Per-core dataflow (all matmuls contract over the partition dim):
  X^T   : TensorE transpose of the core's [S, D] inputs, tile by tile
  Q^T/K^T/V^T = W^T @ X^T  (lhsT = weight slab, rhs = X^T)
  V~    : V^T transposed back to [keys, hd] with a ones column appended
  S^T   : K @ Q^T per head  ->  PSUM [keys, q]
  P     : exp(S^T / 8)      ->  ScalarE activation, PSUM -> SBUF
  U~    : V~^T @ P = [[P V]^T ; colsums]  (softmax numerator + denominator)
  O^T   : U~ rows 0..63 scaled by 1/denominator
  out   : O^T used as lhsT against Wo rows -> partial [S, D]
"""

import numpy as np

import concourse.bass as bass
import concourse.mybir as mybir
import concourse.tile as tile
from concourse import bacc
from concourse.bass import ts, ds
from concourse.bass_utils import run_bass_kernel_spmd
from concourse.masks import make_identity

F32 = mybir.dt.float32
F32R = mybir.dt.float32r

B, S, D = 2, 2048, 1024
H_TOT, HD = 16, 64
HC = 4                 # heads per core
DC = HC * HD           # 256 columns of QKV proj per core
NCORES = 8
P = 128
NDT = D // P           # 8 d-model tiles
NKT = S // P           # 16 key tiles
NQT = S // P           # 16 query tiles
CG = 1024              # q chunk width in attention
NCG = S // CG
SCALE = 1.0 / np.sqrt(HD)

# fp32 matmuls run at 4 cycles/row on the PE; float32r (same bits, reduced
# internal precision) runs at 1 cycle/row for moving dims >= 256. The BIR
# verifier requires every producer of an fp32r matmul operand to emit
# fp32r-rounded data, so the tiles feeding matmuls are typed float32r and
# the evacuation/copy ops do the rounding.
USE_F32R = True
MMDT = F32R if USE_F32R else F32


def _body(ctx, tc, xq, xk, xv, wq, wk, wv, bq, bk, bv, wo, outp):
    nc = tc.nc
    # DRAM bounce buffer for broadcasting softmax denominators across partitions
    rec_dram = nc.dram_tensor("rec_scratch", [8, CG], F32).ap()

    singles = ctx.enter_context(tc.tile_pool(name="singles", bufs=1))
    xpool = ctx.enter_context(tc.tile_pool(name="xpool", bufs=2))
    wpool = ctx.enter_context(tc.tile_pool(name="wpool", bufs=1))
    ppool = ctx.enter_context(tc.tile_pool(name="ppool", bufs=2))
    opool = ctx.enter_context(tc.tile_pool(name="opool", bufs=2))
    psA = ctx.enter_context(tc.tile_pool(name="psA", bufs=2, space="PSUM"))
    psU = ctx.enter_context(tc.tile_pool(name="psU", bufs=1, space="PSUM"))

    identity = singles.tile([P, P], F32, tag="ident")
    make_identity(nc, identity)

    # Persistent per-core tensors (partition dim x free dims)
    XT = singles.tile([P, NDT, S], MMDT, tag="xt")            # X^T, reused per input
    QT = [singles.tile([P, S], MMDT, tag=f"qt{m}", name=f"qt{m}") for m in range(2)]
    KT = [singles.tile([P, S], MMDT, tag=f"kt{m}", name=f"kt{m}") for m in range(2)]
    VT = [singles.tile([P, S], F32, tag=f"vt{m}", name=f"vt{m}") for m in range(2)]  # feeds only PE transposes; rounding happens at the Vt copy
    OT = [singles.tile([P, S], MMDT, tag=f"ot{m}", name=f"ot{m}") for m in range(2)]
    Vt = singles.tile([P, NKT, HC, HD + 1], MMDT, tag="vtile")  # [keys, kt, h, hd+1]

    wo_sb = singles.tile([P, 2, D], MMDT, tag="wo")
    nc.gpsimd.dma_start(out=wo_sb, in_=wo.rearrange("(k p) d -> p k d", p=P))

    # ---- projections: for each input, transpose X then compute W^T @ X^T ----
    for x_dram, w_dram, b_dram, DEST in (
        (xq, wq, bq, QT),
        (xk, wk, bk, KT),
        (xv, wv, bv, VT),
    ):
        w_sb = wpool.tile([P, NDT, DC], MMDT, tag="w")
        nc.gpsimd.dma_start(out=w_sb, in_=w_dram.rearrange("(t p) c -> p t c", p=P))
        b_sb = wpool.tile([P, 2], F32, tag="b")
        nc.sync.dma_start(out=b_sb, in_=b_dram.rearrange("(m p) -> p m", p=P))

        for qt in range(NQT):
            x_sb = xpool.tile([P, D], F32, tag="x")
            nc.sync.dma_start(out=x_sb, in_=x_dram[ts(qt, P), :])
            tp = psA.tile([P, D], F32, tag="mm")
            for dt in range(NDT):
                nc.tensor.transpose(tp[:, ts(dt, P)], x_sb[:, ts(dt, P)], identity)
            nc.vector.tensor_copy(
                out=XT[:, :, ts(qt, P)],
                in_=tp.rearrange("p (t q) -> p t q", q=P),
            )

        for m in range(2):
            for cg in range(NCG):
                ps = psA.tile([P, CG], F32, tag="mm")
                for dt in range(NDT):
                    for c2 in range(2):
                        nc.tensor.matmul(
                            ps[:, ts(c2, 512)],
                            lhsT=w_sb[:, dt, ts(m, P)],
                            rhs=XT[:, dt, ds(cg * CG + c2 * 512, 512)],
                            start=(dt == 0),
                            stop=(dt == NDT - 1),
                        )
                nc.vector.tensor_scalar_add(
                    out=DEST[m][:, ts(cg, CG)], in0=ps, scalar1=b_sb[:, m : m + 1]
                )

    # ---- build V~ = [V | ones] in [keys, hd+1] layout per (kt, head) ----
    # (memset can't emit float32r; stage in f32 and cast-copy)
    ones_f32 = singles.tile([P, NKT, HC, 1], F32, tag="ones")
    nc.vector.memset(ones_f32, 1.0)
    nc.vector.tensor_copy(out=Vt[:, :, :, HD : HD + 1], in_=ones_f32)
    for h in range(HC):
        m, po = divmod(h, 2)
        for g in range(2):
            tp = psA.tile([P, 8 * HD], F32, tag="mm")
            for j in range(8):
                kt = g * 8 + j
                nc.tensor.transpose(
                    tp[:, ts(j, HD)],
                    VT[m][64 * po : 64 * po + 64, ts(kt, P)],
                    identity[64 * po : 64 * po + 64, 64 * po : 64 * po + 64],
                )
            nc.vector.tensor_copy(
                out=Vt[:, ds(g * 8, 8), h, 0:HD],
                in_=tp.rearrange("p (j q) -> p j q", q=HD),
            )

    # ---- attention: per head pair, per q chunk ----
    for hp in range(2):
        for cg in range(NCG):
            U = [psU.tile([HD + 1, CG], F32, tag=f"u{i}", name=f"u{i}") for i in range(2)]
            for kt in range(NKT):
                for i in range(2):
                    h = 2 * hp + i
                    po = 64 * i
                    s_ps = psA.tile([P, CG], F32, tag="mm")
                    for c2 in range(2):
                        nc.tensor.matmul(
                            s_ps[:, ts(c2, 512)],
                            lhsT=KT[hp][po : po + 64, ts(kt, P)],
                            rhs=QT[hp][po : po + 64, ds(cg * CG + c2 * 512, 512)],
                            start=True,
                            stop=True,
                        )
                    pe = ppool.tile([P, CG], MMDT, tag="pexp")
                    nc.scalar.activation(
                        out=pe,
                        in_=s_ps,
                        func=mybir.ActivationFunctionType.Exp,
                        scale=float(SCALE),
                    )
                    for c2 in range(2):
                        nc.tensor.matmul(
                            U[i][:, ts(c2, 512)],
                            lhsT=Vt[:, kt, h, :],
                            rhs=pe[:, ts(c2, 512)],
                            start=(kt == 0),
                            stop=(kt == NKT - 1),
                        )
            for i in range(2):
                idx = (hp * NCG + cg) * 2 + i
                rec = opool.tile([1, CG], F32, tag="rec")
                nc.vector.reciprocal(out=rec, in_=U[i][HD : HD + 1, :])
                nc.sync.dma_start(out=rec_dram[idx : idx + 1, :], in_=rec)
                bc = opool.tile([64, CG], F32, tag="bc")
                row = rec_dram[idx, :]
                bcast = bass.AP(
                    tensor=row.tensor, offset=row.offset, ap=[[0, 64]] + list(row.ap)
                )
                nc.gpsimd.dma_start(out=bc, in_=bcast)
                nc.vector.tensor_mul(
                    out=OT[hp][64 * i : 64 * i + 64, ts(cg, CG)],
                    in0=U[i][0:HD, :],
                    in1=bc,
                )

    # ---- out projection: partial = O @ Wo_slice ----
    for qt in range(NQT):
        ps = psA.tile([P, D], F32, tag="mm")
        for k2 in range(2):
            for c2 in range(2):
                nc.tensor.matmul(
                    ps[:, ts(c2, 512)],
                    lhsT=OT[k2][:, ts(qt, P)],
                    rhs=wo_sb[:, k2, ts(c2, 512)],
                    start=(k2 == 0),
                    stop=(k2 == 1),
                )
        ob = opool.tile([P, D], F32, tag="ob")
        nc.vector.tensor_copy(out=ob, in_=ps)
        nc.sync.dma_start(out=outp[ts(qt, P), :], in_=ob)


def build_nc():
    nc = bacc.Bacc("TRN2", target_bir_lowering=False, debug=False)
    aps = {}
    for name, shape in (
        ("xq", [S, D]),
        ("xk", [S, D]),
        ("xv", [S, D]),
        ("wq", [D, DC]),
        ("wk", [D, DC]),
        ("wv", [D, DC]),
        ("bq", [DC]),
        ("bk", [DC]),
        ("bv", [DC]),
        ("wo", [DC, D]),
    ):
        aps[name] = nc.dram_tensor(name, shape, F32, kind="ExternalInput").ap()
    aps["outp"] = nc.dram_tensor("out_partial", [S, D], F32, kind="ExternalOutput").ap()

    from contextlib import ExitStack

    with tile.TileContext(nc) as tc:
        with ExitStack() as ctx:
            _body(
                ctx,
                tc,
                aps["xq"], aps["xk"], aps["xv"],
                aps["wq"], aps["wk"], aps["wv"],
                aps["bq"], aps["bk"], aps["bv"],
                aps["wo"], aps["outp"],
            )
    nc.compile()
    return nc


def make_in_maps(inputs):
    q = np.asarray(inputs["query"], dtype=np.float32)
    k = np.asarray(inputs.get("key_", inputs.get("key")), dtype=np.float32)
    v = np.asarray(inputs["value"], dtype=np.float32)
    Wq = np.asarray(inputs["Wq"], dtype=np.float32)
    Wk = np.asarray(inputs["Wk"], dtype=np.float32)
    Wv = np.asarray(inputs["Wv"], dtype=np.float32)
    bq = np.asarray(inputs["bq"], dtype=np.float32)
    bk = np.asarray(inputs["bk"], dtype=np.float32)
    bv = np.asarray(inputs["bv"], dtype=np.float32)
    Wo = np.asarray(inputs["Wo"], dtype=np.float32)

    in_maps = []
    for c in range(NCORES):
        b, g = divmod(c, 4)
        cs = slice(DC * g, DC * (g + 1))
        in_maps.append(
            {
                "xq": np.ascontiguousarray(q[b]),
                "xk": np.ascontiguousarray(k[b]),
                "xv": np.ascontiguousarray(v[b]),
                "wq": np.ascontiguousarray(Wq[:, cs]),
                "wk": np.ascontiguousarray(Wk[:, cs]),
                "wv": np.ascontiguousarray(Wv[:, cs]),
                "bq": np.ascontiguousarray(bq[cs]),
                "bk": np.ascontiguousarray(bk[cs]),
                "bv": np.ascontiguousarray(bv[cs]),
                "wo": np.ascontiguousarray(Wo[cs, :]),
            }
        )
    return in_maps


_NC_CACHE = {}


def get_nc():
    if "nc" not in _NC_CACHE:
        _NC_CACHE["nc"] = build_nc()
    return _NC_CACHE["nc"]


def kernel(**inputs):
    nc = get_nc()
    in_maps = make_in_maps(inputs)
    res = run_bass_kernel_spmd(nc, in_maps, list(range(NCORES))).results
    bo = np.asarray(inputs["bo"], dtype=np.float32)
    out = np.empty((B, S, D), dtype=np.float32)
    for b in range(B):
        acc = res[4 * b + 0]["out_partial"].astype(np.float32)
        for g in range(1, 4):
            acc = acc + res[4 * b + g]["out_partial"]
        out[b] = acc + bo[None, :]
    return out


# revision 13
# speedup vs baseline: 1.6044x; 1.0930x over previous
"""Multi-head attention (B=2, S=2048, D=1024, H=16) on 8 Trainium2 cores.

Sharding: core c -> batch b = c // 4, head group g = c % 4 (4 heads each).
Each core computes its 4 heads end-to-end (QKV proj -> attention -> out-proj
partial) and returns a partial [S, D] output; the host sums the 4 partials
per batch and adds the output bias.

Per-core dataflow (all matmuls contract over the partition dim):
  X^T   : TensorE transpose of the core's [S, D] inputs, tile by tile
  Q^T/K^T/V^T = W^T @ X^T  (lhsT = weight slab, rhs = X^T)
  V~    : V^T transposed back to [keys, hd] with a ones column appended
  S^T   : K @ Q^T per head  ->  PSUM [keys, q]
  P     : exp(S^T / 8)      ->  ScalarE activation, PSUM -> SBUF
  U~    : V~^T @ P = [[P V]^T ; colsums]  (softmax numerator + denominator)
  O^T   : U~ rows 0..63 scaled by 1/denominator
  out   : O^T used as lhsT against Wo rows -> partial [S, D]
"""

import numpy as np

import concourse.bass as bass
import concourse.mybir as mybir
import concourse.tile as tile
from concourse import bacc
from concourse.bass import ts, ds
from concourse.bass_utils import run_bass_kernel_spmd
from concourse.masks import make_identity

F32 = mybir.dt.float32
F32R = mybir.dt.float32r

B, S, D = 2, 2048, 1024
H_TOT, HD = 16, 64
HC = 4                 # heads per core
DC = HC * HD           # 256 columns of QKV proj per core
NCORES = 8
P = 128
NDT = D // P           # 8 d-model tiles
NKT = S // P           # 16 key tiles
NQT = S // P           # 16 query tiles
CG = 1024              # q chunk width in attention
NCG = S // CG
SCALE = 1.0 / np.sqrt(HD)

# fp32 matmuls run at 4 cycles/row on the PE; float32r (same bits, reduced
# internal precision) runs at 1 cycle/row for moving dims >= 256. The BIR
# verifier requires every producer of an fp32r matmul operand to emit
# fp32r-rounded data, so the tiles feeding matmuls are typed float32r and
# the evacuation/copy ops do the rounding.
USE_F32R = True
MMDT = F32R if USE_F32R else F32


def _body(ctx, tc, xq, xk, xv, wq, wk, wv, bq, bk, bv, wo, outp):
    nc = tc.nc
    # DRAM bounce buffer for broadcasting softmax denominators across partitions
    rec_dram = nc.dram_tensor("rec_scratch", [8, CG], F32).ap()

    singles = ctx.enter_context(tc.tile_pool(name="singles", bufs=1))
    xpool = ctx.enter_context(tc.tile_pool(name="xpool", bufs=2))
    wpool = ctx.enter_context(tc.tile_pool(name="wpool", bufs=1))
    ppool = ctx.enter_context(tc.tile_pool(name="ppool", bufs=2))
    opool = ctx.enter_context(tc.tile_pool(name="opool", bufs=2))
    psA = ctx.enter_context(tc.tile_pool(name="psA", bufs=2, space="PSUM"))
    psU = ctx.enter_context(tc.tile_pool(name="psU", bufs=1, space="PSUM"))

    identity = singles.tile([P, P], F32, tag="ident")
    make_identity(nc, identity)

    # Persistent per-core tensors (partition dim x free dims)
    XT = singles.tile([P, NDT, S], MMDT, tag="xt")            # X^T, reused per input
    QT = [singles.tile([P, S], MMDT, tag=f"qt{m}", name=f"qt{m}") for m in range(2)]
    KT = [singles.tile([P, S], MMDT, tag=f"kt{m}", name=f"kt{m}") for m in range(2)]
    VT = [singles.tile([P, S], F32, tag=f"vt{m}", name=f"vt{m}") for m in range(2)]  # feeds only PE transposes; rounding happens at the Vt copy
    OT = [singles.tile([P, S], MMDT, tag=f"ot{m}", name=f"ot{m}") for m in range(2)]
    Vt = singles.tile([P, NKT, HC, HD + 1], MMDT, tag="vtile")  # [keys, kt, h, hd+1]

    wo_sb = singles.tile([P, 2, D], MMDT, tag="wo")
    nc.gpsimd.dma_start(out=wo_sb, in_=wo.rearrange("(k p) d -> p k d", p=P))

    # ---- projections: for each input, transpose X then compute W^T @ X^T ----
    # v first, then k, then q: attention needs Vt/KT fully, while QT is
    # consumed per q-chunk, so the scheduler can overlap late q-proj matmuls
    # with early attention chunks.
    for x_dram, w_dram, b_dram, DEST in (
        (xv, wv, bv, VT),
        (xk, wk, bk, KT),
        (xq, wq, bq, QT),
    ):
        w_sb = wpool.tile([P, NDT, DC], MMDT, tag="w")
        nc.gpsimd.dma_start(out=w_sb, in_=w_dram.rearrange("(t p) c -> p t c", p=P))
        b_sb = wpool.tile([P, 2], F32, tag="b")
        nc.sync.dma_start(out=b_sb, in_=b_dram.rearrange("(m p) -> p m", p=P))

        for qt in range(NQT):
            x_sb = xpool.tile([P, D], F32, tag="x")
            nc.sync.dma_start(out=x_sb, in_=x_dram[ts(qt, P), :])
            tp = psA.tile([P, D], F32, tag="mm")
            for dt in range(NDT):
                nc.tensor.transpose(tp[:, ts(dt, P)], x_sb[:, ts(dt, P)], identity)
            nc.vector.tensor_copy(
                out=XT[:, :, ts(qt, P)],
                in_=tp.rearrange("p (t q) -> p t q", q=P),
            )

        for cg in range(NCG):
            for m in range(2):
                ps = psA.tile([P, CG], F32, tag="mm")
                for dt in range(NDT):
                    for c2 in range(2):
                        nc.tensor.matmul(
                            ps[:, ts(c2, 512)],
                            lhsT=w_sb[:, dt, ts(m, P)],
                            rhs=XT[:, dt, ds(cg * CG + c2 * 512, 512)],
                            start=(dt == 0),
                            stop=(dt == NDT - 1),
                        )
                nc.vector.tensor_scalar_add(
                    out=DEST[m][:, ts(cg, CG)], in0=ps, scalar1=b_sb[:, m : m + 1]
                )

    # ---- build V~ = [V | ones] in [keys, hd+1] layout per (kt, head) ----
    # (memset can't emit float32r; stage in f32 and cast-copy)
    ones_f32 = singles.tile([P, NKT, HC, 1], F32, tag="ones")
    nc.vector.memset(ones_f32, 1.0)
    nc.vector.tensor_copy(out=Vt[:, :, :, HD : HD + 1], in_=ones_f32)
    for h in range(HC):
        m, po = divmod(h, 2)
        for g in range(2):
            tp = psA.tile([P, 8 * HD], F32, tag="mm")
            for j in range(8):
                kt = g * 8 + j
                nc.tensor.transpose(
                    tp[:, ts(j, HD)],
                    VT[m][64 * po : 64 * po + 64, ts(kt, P)],
                    identity[64 * po : 64 * po + 64, 64 * po : 64 * po + 64],
                )
            nc.vector.tensor_copy(
                out=Vt[:, ds(g * 8, 8), h, 0:HD],
                in_=tp.rearrange("p (j q) -> p j q", q=HD),
            )

    # ---- attention: per q chunk, per head pair ----
    # Inner kt step: both heads' score matmuls first (they occupy disjoint
    # PE row groups 0-63 / 64-127 and run concurrently), then the exps, then
    # the PV matmuls. U is evacuated to SBUF immediately so the PSUM bank
    # frees up, and the softmax denominator is broadcast via a DRAM bounce
    # before the reciprocal so the reciprocal runs on 64 lanes, not 1.
    for cg in range(NCG):
        for hp in range(2):
            U = [psU.tile([HD + 1, CG], F32, tag=f"u{i}", name=f"u{i}") for i in range(2)]
            for kt in range(NKT):
                s_tiles = []
                for i in range(2):
                    po = 64 * i
                    s_ps = psA.tile([P, CG], F32, tag="mm", name=f"s{i}")
                    for c2 in range(2):
                        nc.tensor.matmul(
                            s_ps[:, ts(c2, 512)],
                            lhsT=KT[hp][po : po + 64, ts(kt, P)],
                            rhs=QT[hp][po : po + 64, ds(cg * CG + c2 * 512, 512)],
                            start=True,
                            stop=True,
                        )
                    s_tiles.append(s_ps)
                p_tiles = []
                for i in range(2):
                    pe = ppool.tile([P, CG], MMDT, tag="pexp", name=f"pexp{i}")
                    nc.scalar.activation(
                        out=pe,
                        in_=s_tiles[i],
                        func=mybir.ActivationFunctionType.Exp,
                        scale=float(SCALE),
                    )
                    p_tiles.append(pe)
                for i in range(2):
                    h = 2 * hp + i
                    for c2 in range(2):
                        nc.tensor.matmul(
                            U[i][:, ts(c2, 512)],
                            lhsT=Vt[:, kt, h, :],
                            rhs=p_tiles[i][:, ts(c2, 512)],
                            start=(kt == 0),
                            stop=(kt == NKT - 1),
                        )
            for i in range(2):
                idx = (hp * NCG + cg) * 2 + i
                usb = opool.tile([HD + 1, CG], F32, tag="usb")
                nc.vector.tensor_copy(out=usb, in_=U[i])
                nc.sync.dma_start(out=rec_dram[idx : idx + 1, :], in_=usb[HD : HD + 1, :])
                bc = opool.tile([64, CG], F32, tag="bc")
                row = rec_dram[idx, :]
                bcast = bass.AP(
                    tensor=row.tensor, offset=row.offset, ap=[[0, 64]] + list(row.ap)
                )
                nc.gpsimd.dma_start(out=bc, in_=bcast)
                nc.vector.reciprocal(out=bc, in_=bc)
                nc.vector.tensor_mul(
                    out=OT[hp][64 * i : 64 * i + 64, ts(cg, CG)],
                    in0=usb[0:HD, :],
                    in1=bc,
                )

    # ---- out projection: partial = O @ Wo_slice ----
    for qt in range(NQT):
        ps = psA.tile([P, D], F32, tag="mm")
        for k2 in range(2):
            for c2 in range(2):
                nc.tensor.matmul(
                    ps[:, ts(c2, 512)],
                    lhsT=OT[k2][:, ts(qt, P)],
                    rhs=wo_sb[:, k2, ts(c2, 512)],
                    start=(k2 == 0),
                    stop=(k2 == 1),
                )
        ob = opool.tile([P, D], F32, tag="ob")
        nc.vector.tensor_copy(out=ob, in_=ps)
        nc.sync.dma_start(out=outp[ts(qt, P), :], in_=ob)


def build_nc():
    nc = bacc.Bacc("TRN2", target_bir_lowering=False, debug=False)
    aps = {}
    for name, shape in (
        ("xq", [S, D]),
        ("xk", [S, D]),
        ("xv", [S, D]),
        ("wq", [D, DC]),
        ("wk", [D, DC]),
        ("wv", [D, DC]),
        ("bq", [DC]),
        ("bk", [DC]),
        ("bv", [DC]),
        ("wo", [DC, D]),
    ):
        aps[name] = nc.dram_tensor(name, shape, F32, kind="ExternalInput").ap()
    aps["outp"] = nc.dram_tensor("out_partial", [S, D], F32, kind="ExternalOutput").ap()

    from contextlib import ExitStack

    with tile.TileContext(nc) as tc:
        with ExitStack() as ctx:
            _body(
                ctx,
                tc,
                aps["xq"], aps["xk"], aps["xv"],
                aps["wq"], aps["wk"], aps["wv"],
                aps["bq"], aps["bk"], aps["bv"],
                aps["wo"], aps["outp"],
            )
    nc.compile()
    return nc


def make_in_maps(inputs):
    q = np.asarray(inputs["query"], dtype=np.float32)
    k = np.asarray(inputs.get("key_", inputs.get("key")), dtype=np.float32)
    v = np.asarray(inputs["value"], dtype=np.float32)
    Wq = np.asarray(inputs["Wq"], dtype=np.float32)
    Wk = np.asarray(inputs["Wk"], dtype=np.float32)
    Wv = np.asarray(inputs["Wv"], dtype=np.float32)
    bq = np.asarray(inputs["bq"], dtype=np.float32)
    bk = np.asarray(inputs["bk"], dtype=np.float32)
    bv = np.asarray(inputs["bv"], dtype=np.float32)
    Wo = np.asarray(inputs["Wo"], dtype=np.float32)

    in_maps = []
    for c in range(NCORES):
        b, g = divmod(c, 4)
        cs = slice(DC * g, DC * (g + 1))
        in_maps.append(
            {
                "xq": np.ascontiguousarray(q[b]),
                "xk": np.ascontiguousarray(k[b]),
                "xv": np.ascontiguousarray(v[b]),
                "wq": np.ascontiguousarray(Wq[:, cs]),
                "wk": np.ascontiguousarray(Wk[:, cs]),
                "wv": np.ascontiguousarray(Wv[:, cs]),
                "bq": np.ascontiguousarray(bq[cs]),
                "bk": np.ascontiguousarray(bk[cs]),
                "bv": np.ascontiguousarray(bv[cs]),
                "wo": np.ascontiguousarray(Wo[cs, :]),
            }
        )
    return in_maps


_NC_CACHE = {}


def get_nc():
    if "nc" not in _NC_CACHE:
        _NC_CACHE["nc"] = build_nc()
    return _NC_CACHE["nc"]


def kernel(**inputs):
    nc = get_nc()
    in_maps = make_in_maps(inputs)
    res = run_bass_kernel_spmd(nc, in_maps, list(range(NCORES))).results
    bo = np.asarray(inputs["bo"], dtype=np.float32)
    out = np.empty((B, S, D), dtype=np.float32)
    for b in range(B):
        acc = res[4 * b + 0]["out_partial"].astype(np.float32)
        for g in range(1, 4):
            acc = acc + res[4 * b + g]["out_partial"]
        out[b] = acc + bo[None, :]
    return out
